# revision 13
# baseline (speedup 1.0000x reference)
"""Trainium2 Bass kernel: cosine-similarity softmin retrieval (DSDM).

reference:  qn = q/||q||; an = a/||a||; sims = qn @ an^T            [B, N]
            w = softmax(10*sims) over N  (softmin of (1-sims)/0.1)
            out = (w @ A)                                           [B, D]

Strategy (8 NeuronCores, flash-attention-style split over N):
  - addresses [200000, 512] sharded row-wise, 25000 rows/core.
  - each core streams its shard once in 128-row tiles (bf16 on-chip, cast
    during the load DMA). Per quad of 4 tiles:
      * A^T chunks via 16 PE transposes -> at_ps [128d, 4c, 512n] (bf16 PSUM)
        -> one DVE copy to SBUF
      * s_raw [64b, 512n] via 4 PSUM-accumulated N=512 matmuls
        (lhsT = qn^T chunk, rhs = at chunk)
      * s_sc = s_raw * (10/||a||) on GPSIMD (inv broadcast along partitions
        via DRAM scratch roundtrip, per quad)
      * w = Exp(s_sc - 10) on ACT, accum_out -> per-quad wsum column
        (fixed shift: cos<=1 so logit-10 <= 0; no running max needed)
      * w^T via 4 PE transposes + DVE copy
      * acc [64, 512] += w^T.T @ A in PSUM across all tiles
  - row norms ss = sum(a^2) split DVE (affine_mul_reduce) / ACT (Square)
  - 10/||a|| = exp(-0.5*ln(ss + eps) + ln10) on ACT (one table set)
  - host: out = sum_c acc_c / sum_c l_c   (gather/unshard + tiny divide)

Padding: per-core row count 25000 = 195*128 + 40; the last tile's 88 pad
rows are zeroed; their s_sc is 0 so they get weight exp(-10), subtracted
exactly on the host.
"""

import math
import os
from collections import OrderedDict

import numpy as np

import concourse.bass as bass
import concourse.tile as tile
from concourse import bacc, mybir
from concourse.bass_utils import run_bass_kernel_spmd
from concourse.masks import make_identity

DT = mybir.dt
AF = mybir.ActivationFunctionType
ALU = mybir.AluOpType

B = 64
D = 512
N_FULL = 200000
NCORES = 8
NPC = N_FULL // NCORES  # 25000
P = 128
LN10 = math.log(10.0)

G = int(os.environ.get("KERNEL_G", "14"))  # tiles per DMA slab
NORM_DVE_OF8 = int(os.environ.get("KERNEL_NORM_DVE_OF8", "3"))  # tiles/8 on DVE
SLAB_BUFS = int(os.environ.get("KERNEL_SLAB_BUFS", "5"))
NORM_AHEAD = int(os.environ.get("KERNEL_NORM_AHEAD", "5"))  # quads of norm lead
MULT_ENGINE = os.environ.get("KERNEL_MULT", "dve")
BACK_DEPTH = int(os.environ.get("KERNEL_BACK_DEPTH", "2"))

LAST_RESULTS = None  # test harness reads exec_time_ns from here


def _patch_act_tables():
    """Prefer the combined natural_log_exp set so Ln/Exp/Square/Copy share
    one ACT table load instead of thrashing 2 loads per slab (~2.7us each)."""
    if getattr(bacc.get_activation_tables, "_patched", False):
        return
    orig = bacc.get_activation_tables

    keep = {AF.Ln, AF.Exp, AF.Square}

    def patched(arch):
        tabs = orig(arch)
        out = OrderedDict()
        for k, fns in tabs.items():
            if k == "natural_log_exp_and_others":
                out[k] = fns
            else:
                out[k] = {f for f in fns if f not in keep}
        return out

    patched._patched = True
    bacc.get_activation_tables = patched


def _build(npc=NPC):
    _patch_act_tables()
    ntiles = (npc + P - 1) // P
    assert ntiles % 4 == 0
    nquads = ntiles // 4
    g = G
    nslabs = (ntiles + g - 1) // g
    real_last = npc - (ntiles - 1) * P  # rows in final tile

    nc = bacc.Bacc("TRN2")
    q_d = nc.dram_tensor("query", [B, D], DT.float32, kind="ExternalInput")
    a_d = nc.dram_tensor("addresses", [npc, D], DT.float32, kind="ExternalInput")
    acc_d = nc.dram_tensor("acc", [B, D], DT.float32, kind="ExternalOutput")
    lsum_d = nc.dram_tensor("lsum", [B, nquads], DT.float32, kind="ExternalOutput")

    with tile.TileContext(nc) as tc:
        with (
            tc.tile_pool(name="const", bufs=1) as const,
            tc.tile_pool(name="slab", bufs=SLAB_BUFS) as slab_pool,
            tc.tile_pool(name="at", bufs=2) as at_pool,
            tc.tile_pool(name="wt", bufs=4) as wt_pool,
            tc.tile_pool(name="ssc", bufs=2) as ssc_pool,
            tc.tile_pool(name="small", bufs=4) as small,
            tc.tile_pool(name="ps_at", bufs=1, space="PSUM") as ps_at,
            tc.tile_pool(name="ps_s", bufs=2, space="PSUM") as ps_s,
            tc.tile_pool(name="ps_wt", bufs=1, space="PSUM") as ps_wt,
            tc.tile_pool(name="ps_acc", bufs=1, space="PSUM") as ps_acc,
            tc.tile_pool(name="dram", bufs=1, space="DRAM") as dram_pool,
        ):
            ident = const.tile([P, P], DT.bfloat16)
            make_identity(nc, ident)
            bias_main = const.tile([B, 1], DT.float32)
            nc.vector.memset(bias_main, -10.0)
            eps12 = const.tile([P, 1], DT.float32)
            nc.vector.memset(eps12, 1e-12)
            ln10b = const.tile([P, 1], DT.float32)
            nc.vector.memset(ln10b, LN10)
            identf = const.tile([P, P], DT.float32)
            make_identity(nc, identf)
            wsums = const.tile([B, nquads], DT.float32)

            # ---- query preprocessing: qn^T bf16 chunks [128d, 4c, 64b] ----
            q_sb = const.tile([B, D], DT.float32)
            nc.sync.dma_start(out=q_sb, in_=q_d[:, :])
            qsq = const.tile([B, D], DT.float32)
            ssq = const.tile([B, 1], DT.float32)
            nc.scalar.activation(qsq, q_sb, AF.Square, accum_out=ssq)
            lnq = const.tile([B, 1], DT.float32)
            nc.scalar.activation(lnq, ssq, AF.Ln, bias=eps12[:B])
            invq = const.tile([B, 1], DT.float32)
            nc.scalar.activation(invq, lnq, AF.Exp, scale=-0.5)
            qn = const.tile([B, D], DT.bfloat16)
            nc.vector.tensor_scalar_mul(out=qn, in0=q_sb, scalar1=invq)
            qnT = const.tile([P, 4, B], DT.bfloat16)
            for c in range(4):
                qt_ps = ps_wt.tile([P, B], DT.bfloat16, tag="qprep")
                nc.tensor.transpose(qt_ps, qn[:, c * P:(c + 1) * P], ident[:B, :B])
                nc.scalar.copy(qnT[:, c, :], qt_ps)

            # ---- main streaming loop ----
            acc_ps = ps_acc.tile([B, D], DT.float32)
            scr = dram_pool.tile([1, ntiles * P], DT.float32)
            slab_tiles = {}

            slab_ss = {}
            norms_done = [0]  # tiles with norms emitted (in order)

            def ensure_slab(sg):
                if sg in slab_tiles:
                    return slab_tiles[sg]
                t0, t1 = sg * g, min((sg + 1) * g, ntiles)  # tile range
                gg = t1 - t0
                a_sl = slab_pool.tile([P, gg, D], DT.bfloat16)
                last_slab = t1 == ntiles
                if not last_slab or real_last == P:
                    nc.gpsimd.dma_start(
                        out=a_sl,
                        in_=a_d[t0 * P:t1 * P, :].rearrange(
                            "(t p) d -> p t d", p=P))
                else:
                    for t in range(gg - 1):
                        r0 = (t0 + t) * P
                        nc.gpsimd.dma_start(out=a_sl[:, t, :], in_=a_d[r0:r0 + P, :])
                    nc.gpsimd.memset(a_sl[:, gg - 1, :], 0)
                    nc.gpsimd.dma_start(
                        out=a_sl[:real_last, gg - 1, :],
                        in_=a_d[(ntiles - 1) * P:npc, :])
                slab_tiles[sg] = a_sl
                return a_sl

            def a_tile(gt):
                sg, t = divmod(gt, g)
                return ensure_slab(sg)[:, t, :]

            def norm_tiles_upto(gt_end):
                """Emit per-tile norms (spread across quads) and, at each
                slab's last tile, the inv finalize + scr write."""
                while norms_done[0] < min(gt_end, ntiles):
                    gt = norms_done[0]
                    sg, t = divmod(gt, g)
                    t0, t1 = sg * g, min((sg + 1) * g, ntiles)
                    gg = t1 - t0
                    a_sl = ensure_slab(sg)
                    if t == 0:
                        ss_new = small.tile([P, gg], DT.float32, tag="ss")
                        slab_ss[sg] = ss_new
                    ss = slab_ss[sg]
                    sq = small.tile([P, D], DT.bfloat16, tag="sq")
                    if (gt % 8) < NORM_DVE_OF8:
                        nc.vector.affine_mul_reduce(
                            out=sq, accum_out=ss[:, t:t + 1],
                            in0=a_sl[:, t, :], in1=a_sl[:, t, :], scale=1.0,
                            bias=0.0)
                    else:
                        nc.scalar.activation(sq, a_sl[:, t, :], AF.Square,
                                             accum_out=ss[:, t:t + 1])
                    if t == gg - 1:
                        lns = small.tile([P, gg], DT.float32, tag="lns")
                        nc.scalar.activation(lns, ss, AF.Ln, bias=eps12)
                        inv = small.tile([P, gg], DT.float32, tag="inv")
                        nc.scalar.activation(inv, lns, AF.Exp, scale=-0.5,
                                             bias=ln10b)
                        ivt_ps = ps_wt.tile([g, P], DT.float32, tag="ivt")
                        nc.tensor.transpose(ivt_ps[:gg], inv, identf)
                        ivt = small.tile([g, P], DT.float32, tag="ivt_sb")
                        nc.vector.tensor_copy(ivt[:gg], ivt_ps[:gg])
                        nc.sync.dma_start(
                            out=bass.AP(tensor=scr.tensor,
                                        offset=scr.offset + t0 * P,
                                        ap=[[P, gg], [1, P]]),
                            in_=ivt[:gg])
                    norms_done[0] += 1


            def front_a(q):
                # A^T for the quad: [128d, 4c, 512n] via 16 PE transposes
                at_ps = ps_at.tile([P, 4, 4 * P], DT.bfloat16)
                for t in range(4):
                    a_t = a_tile(4 * q + t)
                    for c in range(4):
                        nc.tensor.transpose(
                            at_ps[:, c, t * P:(t + 1) * P],
                            a_t[:, c * P:(c + 1) * P], ident)
                at_sb = at_pool.tile([P, 4, 4 * P], DT.bfloat16)
                nc.vector.tensor_copy(at_sb, at_ps)
                # inv broadcast [64b, 512n] from DRAM scratch (HWDGE: keeps
                # the Q7 descriptor path free for the big slab loads)
                inv_bc = ssc_pool.tile([B, 4 * P], DT.float32, tag="inv_bc")
                nc.sync.dma_start(
                    out=inv_bc,
                    in_=bass.AP(tensor=scr.tensor,
                                offset=scr.offset + q * 4 * P,
                                ap=[[0, B], [1, 4 * P]]))
                return at_sb, inv_bc

            def front_b(q, at_sb, inv_bc):
                # sims: 4 wide matmuls accumulating over d-chunks
                s_ps = ps_s.tile([B, 4 * P], DT.float32, tag="s")
                for c in range(4):
                    nc.tensor.matmul(
                        s_ps, lhsT=qnT[:, c, :], rhs=at_sb[:, c, :],
                        start=(c == 0), stop=(c == 3))
                s_sc = ssc_pool.tile([B, 4 * P], DT.float32, tag="s_sc")
                nc.vector.tensor_mul(s_sc, s_ps, inv_bc)
                w_q = wt_pool.tile([B, 4 * P], DT.bfloat16, tag="w_q")
                nc.scalar.activation(w_q, s_sc, AF.Exp, bias=bias_main,
                                     accum_out=wsums[:, q:q + 1])
                return w_q

            def stage_back(q, w_q):
                wt_ps = ps_wt.tile([P, 4, B], DT.bfloat16, tag="wt")
                for t in range(4):
                    nc.tensor.transpose(
                        wt_ps[:, t, :], w_q[:, t * P:(t + 1) * P],
                        ident[:B, :B])
                wt_sb = wt_pool.tile([P, 4, B], DT.bfloat16, tag="wt_sb")
                nc.vector.tensor_copy(wt_sb, wt_ps)
                for t in range(4):
                    gt = 4 * q + t
                    nc.tensor.matmul(
                        acc_ps, lhsT=wt_sb[:, t, :], rhs=a_tile(gt),
                        start=(gt == 0), stop=(gt == ntiles - 1))

            # pipeline warmup: norms for the first NORM_AHEAD quads' tiles
            norm_tiles_upto(4 * NORM_AHEAD)
            from collections import deque
            pending = deque()  # (q, w_q) awaiting back stage, depth BACK_DEPTH
            for q in range(nquads):
                # prefetch slab DMAs ahead of norm consumption
                sg_ahead = min((4 * (q + NORM_AHEAD)) // g + 1, nslabs - 1)
                ensure_slab(sg_ahead)
                fr = front_a(q)
                # an older quad's back stage fills the PE while the DVE
                # copies this quad's A^T out of PSUM and the exp drains
                if len(pending) >= BACK_DEPTH:
                    stage_back(*pending.popleft())
                w_q = front_b(q, *fr)
                pending.append((q, w_q))
                # spread future norms behind this quad's critical ops
                norm_tiles_upto(4 * (q + 1 + NORM_AHEAD))
            while pending:
                stage_back(*pending.popleft())

            # ---- epilogue: writeback ----
            acc_sb = const.tile([B, D], DT.float32)
            nc.scalar.copy(acc_sb, acc_ps)
            nc.sync.dma_start(out=acc_d[:, :], in_=acc_sb)
            nc.sync.dma_start(out=lsum_d[:, :], in_=wsums)

    nc.finalize()
    return nc


_NC_CACHE = {}


def _get_nc(npc=NPC):
    if npc not in _NC_CACHE:
        _NC_CACHE[npc] = _build(npc)
    return _NC_CACHE[npc]


def kernel(query, addresses):
    global LAST_RESULTS
    query = np.ascontiguousarray(np.asarray(query), dtype=np.float32)
    addresses = np.ascontiguousarray(np.asarray(addresses), dtype=np.float32)
    n = addresses.shape[0]
    npc = n // NCORES
    assert npc * NCORES == n
    nc = _get_nc(npc)
    in_maps = [
        {"query": query, "addresses": addresses[c * npc:(c + 1) * npc]}
        for c in range(NCORES)
    ]
    res = run_bass_kernel_spmd(nc, in_maps, core_ids=list(range(NCORES)))
    LAST_RESULTS = res
    acc = np.zeros((B, D), np.float64)
    l = np.zeros((B, 1), np.float64)
    ntiles = (npc + P - 1) // P
    n_pad = ntiles * P - npc  # zero rows in the padded last tile
    for r in res.results:
        acc += r["acc"].astype(np.float64)
        l += r["lsum"].astype(np.float64).sum(axis=1, keepdims=True)
        if n_pad:
            # each pad row contributes exactly exp(0*scale - 10)
            l -= n_pad * math.exp(-10.0)
    return (acc / l).astype(np.float32)


# revision 14
# speedup vs baseline: 1.1579x; 1.1579x over previous
"""Trainium2 Bass kernel: cosine-similarity softmin retrieval (DSDM).

reference:  qn = q/||q||; an = a/||a||; sims = qn @ an^T            [B, N]
            w = softmax(10*sims) over N  (softmin of (1-sims)/0.1)
            out = (w @ A)                                           [B, D]

Strategy (8 NeuronCores, flash-attention-style split over N):
  - addresses [200000, 512] sharded row-wise, 25000 rows/core.
  - each core streams its shard once in 128-row tiles (bf16 on-chip, cast
    during the load DMA). Per quad of 4 tiles:
      * A^T chunks via 16 PE transposes -> at_ps [128d, 4c, 512n] (bf16 PSUM)
        -> one DVE copy to SBUF
      * s_raw [64b, 512n] via 4 PSUM-accumulated N=512 matmuls
        (lhsT = qn^T chunk, rhs = at chunk)
      * s_sc = s_raw * (10/||a||) on GPSIMD (inv broadcast along partitions
        via DRAM scratch roundtrip, per quad)
      * w = Exp(s_sc - 10) on ACT, accum_out -> per-quad wsum column
        (fixed shift: cos<=1 so logit-10 <= 0; no running max needed)
      * w^T via 4 PE transposes + DVE copy
      * acc [64, 512] += w^T.T @ A in PSUM across all tiles
  - row norms ss = sum(a^2) split DVE (affine_mul_reduce) / ACT (Square)
  - 10/||a|| = exp(-0.5*ln(ss + eps) + ln10) on ACT (one table set)
  - host: out = sum_c acc_c / sum_c l_c   (gather/unshard + tiny divide)

Padding: per-core row count 25000 = 195*128 + 40; the last tile's 88 pad
rows are zeroed; their s_sc is 0 so they get weight exp(-10), subtracted
exactly on the host.
"""

import math
import os
from collections import OrderedDict

import numpy as np

import concourse.bass as bass
import concourse.tile as tile
from concourse import bacc, mybir
from concourse.bass_utils import run_bass_kernel_spmd
from concourse.masks import make_identity

DT = mybir.dt
AF = mybir.ActivationFunctionType
ALU = mybir.AluOpType

B = 64
D = 512
N_FULL = 200000
NCORES = 8
NPC = N_FULL // NCORES  # 25000
P = 128
LN10 = math.log(10.0)

G = int(os.environ.get("KERNEL_G", "14"))  # tiles per DMA slab
NORM_DVE_OF8 = int(os.environ.get("KERNEL_NORM_DVE_OF8", "3"))  # tiles/8 on DVE
SLAB_BUFS = int(os.environ.get("KERNEL_SLAB_BUFS", "5"))
NORM_AHEAD = int(os.environ.get("KERNEL_NORM_AHEAD", "5"))  # quads of norm lead
MULT_ENGINE = os.environ.get("KERNEL_MULT", "dve")
BACK_DEPTH = int(os.environ.get("KERNEL_BACK_DEPTH", "2"))

LAST_RESULTS = None  # test harness reads exec_time_ns from here


def _patch_act_tables():
    """Prefer the combined natural_log_exp set so Ln/Exp/Square/Copy share
    one ACT table load instead of thrashing 2 loads per slab (~2.7us each)."""
    if getattr(bacc.get_activation_tables, "_patched", False):
        return
    orig = bacc.get_activation_tables

    keep = {AF.Ln, AF.Exp, AF.Square}

    def patched(arch):
        tabs = orig(arch)
        out = OrderedDict()
        for k, fns in tabs.items():
            if k == "natural_log_exp_and_others":
                out[k] = fns
            else:
                out[k] = {f for f in fns if f not in keep}
        return out

    patched._patched = True
    bacc.get_activation_tables = patched


def _build(npc=NPC):
    _patch_act_tables()
    ntiles = (npc + P - 1) // P
    assert ntiles % 4 == 0
    nquads = ntiles // 4
    g = G
    nslabs = (ntiles + g - 1) // g
    real_last = npc - (ntiles - 1) * P  # rows in final tile

    nc = bacc.Bacc("TRN2")
    q_d = nc.dram_tensor("query", [B, D], DT.float32, kind="ExternalInput")
    a_d = nc.dram_tensor("addresses", [npc, D], DT.float32, kind="ExternalInput")
    acc_d = nc.dram_tensor("acc", [B, D], DT.float32, kind="ExternalOutput")
    lsum_d = nc.dram_tensor("lsum", [B, nquads], DT.float32, kind="ExternalOutput")

    with tile.TileContext(nc) as tc:
        with (
            tc.tile_pool(name="const", bufs=1) as const,
            tc.tile_pool(name="slab", bufs=SLAB_BUFS) as slab_pool,
            tc.tile_pool(name="at", bufs=2) as at_pool,
            tc.tile_pool(name="wt", bufs=4) as wt_pool,
            tc.tile_pool(name="ssc", bufs=2) as ssc_pool,
            tc.tile_pool(name="small", bufs=4) as small,
            tc.tile_pool(name="ps_at", bufs=1, space="PSUM") as ps_at,
            tc.tile_pool(name="ps_s", bufs=2, space="PSUM") as ps_s,
            tc.tile_pool(name="ps_wt", bufs=1, space="PSUM") as ps_wt,
            tc.tile_pool(name="ps_acc", bufs=1, space="PSUM") as ps_acc,
            tc.tile_pool(name="dram", bufs=1, space="DRAM") as dram_pool,
        ):
            ident = const.tile([P, P], DT.bfloat16)
            make_identity(nc, ident)
            bias_main = const.tile([B, 1], DT.float32)
            nc.vector.memset(bias_main, -10.0)
            eps12 = const.tile([P, 1], DT.float32)
            nc.vector.memset(eps12, 1e-12)
            ln10b = const.tile([P, 1], DT.float32)
            nc.vector.memset(ln10b, LN10)
            identf = const.tile([P, P], DT.float32)
            make_identity(nc, identf)
            wsums = const.tile([B, nquads], DT.float32)

            # ---- query preprocessing: qn^T bf16 chunks [128d, 4c, 64b] ----
            q_sb = const.tile([B, D], DT.float32)
            nc.sync.dma_start(out=q_sb, in_=q_d[:, :])
            qsq = const.tile([B, D], DT.float32)
            ssq = const.tile([B, 1], DT.float32)
            nc.scalar.activation(qsq, q_sb, AF.Square, accum_out=ssq)
            lnq = const.tile([B, 1], DT.float32)
            nc.scalar.activation(lnq, ssq, AF.Ln, bias=eps12[:B])
            invq = const.tile([B, 1], DT.float32)
            nc.scalar.activation(invq, lnq, AF.Exp, scale=-0.5)
            qn = const.tile([B, D], DT.bfloat16)
            nc.vector.tensor_scalar_mul(out=qn, in0=q_sb, scalar1=invq)
            qnT = const.tile([P, 4, B], DT.bfloat16)
            for c in range(4):
                qt_ps = ps_wt.tile([P, B], DT.bfloat16, tag="qprep")
                nc.tensor.transpose(qt_ps, qn[:, c * P:(c + 1) * P], ident[:B, :B])
                nc.scalar.copy(qnT[:, c, :], qt_ps)

            # ---- main streaming loop ----
            acc_ps = ps_acc.tile([B, D], DT.float32)
            scr = dram_pool.tile([1, ntiles * P], DT.float32)
            slab_tiles = {}

            slab_ss = {}
            norms_done = [0]  # tiles with norms emitted (in order)

            def ensure_slab(sg):
                if sg in slab_tiles:
                    return slab_tiles[sg]
                t0, t1 = sg * g, min((sg + 1) * g, ntiles)  # tile range
                gg = t1 - t0
                a_sl = slab_pool.tile([P, gg, D], DT.bfloat16)
                last_slab = t1 == ntiles
                if not last_slab or real_last == P:
                    nc.gpsimd.dma_start(
                        out=a_sl,
                        in_=a_d[t0 * P:t1 * P, :].rearrange(
                            "(t p) d -> p t d", p=P))
                else:
                    for t in range(gg - 1):
                        r0 = (t0 + t) * P
                        nc.gpsimd.dma_start(out=a_sl[:, t, :], in_=a_d[r0:r0 + P, :])
                    nc.gpsimd.memset(a_sl[:, gg - 1, :], 0)
                    nc.gpsimd.dma_start(
                        out=a_sl[:real_last, gg - 1, :],
                        in_=a_d[(ntiles - 1) * P:npc, :])
                slab_tiles[sg] = a_sl
                return a_sl

            def a_tile(gt):
                sg, t = divmod(gt, g)
                return ensure_slab(sg)[:, t, :]

            def norm_tiles_upto(gt_end):
                """Emit per-tile norms (spread across quads) and, at each
                slab's last tile, the inv finalize + scr write."""
                while norms_done[0] < min(gt_end, ntiles):
                    gt = norms_done[0]
                    sg, t = divmod(gt, g)
                    t0, t1 = sg * g, min((sg + 1) * g, ntiles)
                    gg = t1 - t0
                    a_sl = ensure_slab(sg)
                    if t == 0:
                        ss_new = small.tile([P, gg], DT.float32, tag="ss")
                        slab_ss[sg] = ss_new
                    ss = slab_ss[sg]
                    sq = small.tile([P, D], DT.bfloat16, tag="sq")
                    if (gt % 8) < NORM_DVE_OF8:
                        nc.vector.affine_mul_reduce(
                            out=sq, accum_out=ss[:, t:t + 1],
                            in0=a_sl[:, t, :], in1=a_sl[:, t, :], scale=1.0,
                            bias=0.0)
                    else:
                        nc.scalar.activation(sq, a_sl[:, t, :], AF.Square,
                                             accum_out=ss[:, t:t + 1])
                    if t == gg - 1:
                        lns = small.tile([P, gg], DT.float32, tag="lns")
                        nc.scalar.activation(lns, ss, AF.Ln, bias=eps12)
                        inv = small.tile([P, gg], DT.float32, tag="inv")
                        nc.scalar.activation(inv, lns, AF.Exp, scale=-0.5,
                                             bias=ln10b)
                        ivt_ps = ps_wt.tile([g, P], DT.float32, tag="ivt")
                        nc.tensor.transpose(ivt_ps[:gg], inv, identf)
                        ivt = small.tile([g, P], DT.float32, tag="ivt_sb")
                        nc.vector.tensor_copy(ivt[:gg], ivt_ps[:gg])
                        nc.sync.dma_start(
                            out=bass.AP(tensor=scr.tensor,
                                        offset=scr.offset + t0 * P,
                                        ap=[[P, gg], [1, P]]),
                            in_=ivt[:gg])
                    norms_done[0] += 1


            def front_a(q):
                # A^T for the quad: [128d, 4c, 512n] via 16 PE transposes
                at_ps = ps_at.tile([P, 4, 4 * P], DT.bfloat16)
                for t in range(4):
                    a_t = a_tile(4 * q + t)
                    for c in range(4):
                        nc.tensor.transpose(
                            at_ps[:, c, t * P:(t + 1) * P],
                            a_t[:, c * P:(c + 1) * P], ident)
                at_sb = at_pool.tile([P, 4, 4 * P], DT.bfloat16)
                nc.vector.tensor_copy(at_sb, at_ps)
                # inv broadcast [64b, 512n] from DRAM scratch (HWDGE: keeps
                # the Q7 descriptor path free for the big slab loads)
                inv_bc = ssc_pool.tile([B, 4 * P], DT.float32, tag="inv_bc")
                nc.sync.dma_start(
                    out=inv_bc,
                    in_=bass.AP(tensor=scr.tensor,
                                offset=scr.offset + q * 4 * P,
                                ap=[[0, B], [1, 4 * P]]))
                return at_sb, inv_bc

            def front_b(q, at_sb, inv_bc):
                # sims: 4 wide matmuls accumulating over d-chunks
                s_ps = ps_s.tile([B, 4 * P], DT.float32, tag="s")
                for c in range(4):
                    nc.tensor.matmul(
                        s_ps, lhsT=qnT[:, c, :], rhs=at_sb[:, c, :],
                        start=(c == 0), stop=(c == 3))
                s_sc = ssc_pool.tile([B, 4 * P], DT.float32, tag="s_sc")
                nc.vector.tensor_mul(s_sc, s_ps, inv_bc)
                w_q = wt_pool.tile([B, 4 * P], DT.bfloat16, tag="w_q")
                nc.scalar.activation(w_q, s_sc, AF.Exp, bias=bias_main,
                                     accum_out=wsums[:, q:q + 1])
                return w_q

            def stage_back(q, w_q):
                wt_ps = ps_wt.tile([P, 4, B], DT.bfloat16, tag="wt")
                for t in range(4):
                    nc.tensor.transpose(
                        wt_ps[:, t, :], w_q[:, t * P:(t + 1) * P],
                        ident[:B, :B])
                wt_sb = wt_pool.tile([P, 4, B], DT.bfloat16, tag="wt_sb")
                nc.vector.tensor_copy(wt_sb, wt_ps)
                for t in range(4):
                    gt = 4 * q + t
                    nc.tensor.matmul(
                        acc_ps, lhsT=wt_sb[:, t, :], rhs=a_tile(gt),
                        start=(gt == 0), stop=(gt == ntiles - 1))

            # Software pipeline, skewed so every cross-engine dependency
            # has a full iteration of slack (no FIFO priority inversions):
            #   iter i emits: front_a(i) | back(i-2) | front_b(i-1)
            # PE stream/iter:  transp(i), wT(i-2), acc(i-2), sims(i-1)
            # DVE stream/iter: at-copy(i), wt-copy(i-2), mult(i-1), norms
            norm_tiles_upto(4 * NORM_AHEAD)
            fr = {}
            wq = {}
            for q in range(nquads + 2):
                if q < nquads:
                    sg_ahead = min((4 * (q + NORM_AHEAD)) // g + 1, nslabs - 1)
                    ensure_slab(sg_ahead)
                    fr[q] = front_a(q)
                if q - 2 >= 0:
                    stage_back(q - 2, wq.pop(q - 2))
                if q - 1 >= 0 and q - 1 < nquads:
                    wq[q - 1] = front_b(q - 1, *fr.pop(q - 1))
                if q < nquads:
                    norm_tiles_upto(4 * (q + 1 + NORM_AHEAD))

            # ---- epilogue: writeback ----
            acc_sb = const.tile([B, D], DT.float32)
            nc.scalar.copy(acc_sb, acc_ps)
            nc.sync.dma_start(out=acc_d[:, :], in_=acc_sb)
            nc.sync.dma_start(out=lsum_d[:, :], in_=wsums)

    nc.finalize()
    return nc


_NC_CACHE = {}


def _get_nc(npc=NPC):
    if npc not in _NC_CACHE:
        _NC_CACHE[npc] = _build(npc)
    return _NC_CACHE[npc]


def kernel(query, addresses):
    global LAST_RESULTS
    query = np.ascontiguousarray(np.asarray(query), dtype=np.float32)
    addresses = np.ascontiguousarray(np.asarray(addresses), dtype=np.float32)
    n = addresses.shape[0]
    npc = n // NCORES
    assert npc * NCORES == n
    nc = _get_nc(npc)
    in_maps = [
        {"query": query, "addresses": addresses[c * npc:(c + 1) * npc]}
        for c in range(NCORES)
    ]
    res = run_bass_kernel_spmd(nc, in_maps, core_ids=list(range(NCORES)))
    LAST_RESULTS = res
    acc = np.zeros((B, D), np.float64)
    l = np.zeros((B, 1), np.float64)
    ntiles = (npc + P - 1) // P
    n_pad = ntiles * P - npc  # zero rows in the padded last tile
    for r in res.results:
        acc += r["acc"].astype(np.float64)
        l += r["lsum"].astype(np.float64).sum(axis=1, keepdims=True)
        if n_pad:
            # each pad row contributes exactly exp(0*scale - 10)
            l -= n_pad * math.exp(-10.0)
    return (acc / l).astype(np.float32)


# revision 15
# speedup vs baseline: 1.2650x; 1.0925x over previous
"""Trainium2 Bass kernel: cosine-similarity softmin retrieval (DSDM).

reference:  qn = q/||q||; an = a/||a||; sims = qn @ an^T            [B, N]
            w = softmax(10*sims) over N  (softmin of (1-sims)/0.1)
            out = (w @ A)                                           [B, D]

Strategy (8 NeuronCores, flash-attention-style split over N):
  - addresses [200000, 512] sharded row-wise, 25000 rows/core.
  - each core streams its shard once in 128-row tiles (bf16 on-chip, cast
    during the load DMA). Per quad of 4 tiles:
      * A^T chunks via 16 PE transposes -> at_ps [128d, 4c, 512n] (bf16 PSUM)
        -> one DVE copy to SBUF
      * s_raw [64b, 512n] via 4 PSUM-accumulated N=512 matmuls
        (lhsT = qn^T chunk, rhs = at chunk)
      * s_sc = s_raw * (10/||a||) on GPSIMD (inv broadcast along partitions
        via DRAM scratch roundtrip, per quad)
      * w = Exp(s_sc - 10) on ACT, accum_out -> per-quad wsum column
        (fixed shift: cos<=1 so logit-10 <= 0; no running max needed)
      * w^T via 4 PE transposes + DVE copy
      * acc [64, 512] += w^T.T @ A in PSUM across all tiles
  - row norms ss = sum(a^2) split DVE (affine_mul_reduce) / ACT (Square)
  - 10/||a|| = exp(-0.5*ln(ss + eps) + ln10) on ACT (one table set)
  - host: out = sum_c acc_c / sum_c l_c   (gather/unshard + tiny divide)

Padding: per-core row count 25000 = 195*128 + 40; the last tile's 88 pad
rows are zeroed; their s_sc is 0 so they get weight exp(-10), subtracted
exactly on the host.
"""

import math
import os
from collections import OrderedDict

import numpy as np

import concourse.bass as bass
import concourse.tile as tile
from concourse import bacc, mybir
from concourse.bass_utils import run_bass_kernel_spmd
from concourse.masks import make_identity

DT = mybir.dt
AF = mybir.ActivationFunctionType
ALU = mybir.AluOpType

B = 64
D = 512
N_FULL = 200000
NCORES = 8
NPC = N_FULL // NCORES  # 25000
P = 128
LN10 = math.log(10.0)

G = int(os.environ.get("KERNEL_G", "14"))  # tiles per DMA slab
NORM_DVE_OF8 = int(os.environ.get("KERNEL_NORM_DVE_OF8", "3"))  # tiles/8 on DVE
SLAB_BUFS = int(os.environ.get("KERNEL_SLAB_BUFS", "6"))
NORM_AHEAD = int(os.environ.get("KERNEL_NORM_AHEAD", "5"))  # quads of norm lead
MULT_ENGINE = os.environ.get("KERNEL_MULT", "dve")
BACK_DEPTH = int(os.environ.get("KERNEL_BACK_DEPTH", "3"))

LAST_RESULTS = None  # test harness reads exec_time_ns from here


def _patch_act_tables():
    """Prefer the combined natural_log_exp set so Ln/Exp/Square/Copy share
    one ACT table load instead of thrashing 2 loads per slab (~2.7us each)."""
    if getattr(bacc.get_activation_tables, "_patched", False):
        return
    orig = bacc.get_activation_tables

    keep = {AF.Ln, AF.Exp, AF.Square}

    def patched(arch):
        tabs = orig(arch)
        out = OrderedDict()
        for k, fns in tabs.items():
            if k == "natural_log_exp_and_others":
                out[k] = fns
            else:
                out[k] = {f for f in fns if f not in keep}
        return out

    patched._patched = True
    bacc.get_activation_tables = patched


def _build(npc=NPC):
    _patch_act_tables()
    ntiles = (npc + P - 1) // P
    assert ntiles % 4 == 0
    nquads = ntiles // 4
    g = G
    nslabs = (ntiles + g - 1) // g
    real_last = npc - (ntiles - 1) * P  # rows in final tile

    nc = bacc.Bacc("TRN2")
    q_d = nc.dram_tensor("query", [B, D], DT.float32, kind="ExternalInput")
    a_d = nc.dram_tensor("addresses", [npc, D], DT.float32, kind="ExternalInput")
    acc_d = nc.dram_tensor("acc", [B, D], DT.float32, kind="ExternalOutput")
    lsum_d = nc.dram_tensor("lsum", [B, nquads], DT.float32, kind="ExternalOutput")

    with tile.TileContext(nc) as tc:
        with (
            tc.tile_pool(name="const", bufs=1) as const,
            tc.tile_pool(name="slab", bufs=SLAB_BUFS) as slab_pool,
            tc.tile_pool(name="at", bufs=2) as at_pool,
            tc.tile_pool(name="wt", bufs=5) as wt_pool,
            tc.tile_pool(name="ssc", bufs=2) as ssc_pool,
            tc.tile_pool(name="small", bufs=4) as small,
            tc.tile_pool(name="ps_at", bufs=1, space="PSUM") as ps_at,
            tc.tile_pool(name="ps_s", bufs=2, space="PSUM") as ps_s,
            tc.tile_pool(name="ps_wt", bufs=1, space="PSUM") as ps_wt,
            tc.tile_pool(name="ps_acc", bufs=1, space="PSUM") as ps_acc,
            tc.tile_pool(name="dram", bufs=1, space="DRAM") as dram_pool,
        ):
            ident = const.tile([P, P], DT.bfloat16)
            make_identity(nc, ident)
            bias_main = const.tile([B, 1], DT.float32)
            nc.vector.memset(bias_main, -10.0)
            eps12 = const.tile([P, 1], DT.float32)
            nc.vector.memset(eps12, 1e-12)
            ln10b = const.tile([P, 1], DT.float32)
            nc.vector.memset(ln10b, LN10)
            identf = const.tile([P, P], DT.float32)
            make_identity(nc, identf)
            wsums = const.tile([B, nquads], DT.float32)

            # ---- query preprocessing: qn^T bf16 chunks [128d, 4c, 64b] ----
            q_sb = const.tile([B, D], DT.float32)
            nc.sync.dma_start(out=q_sb, in_=q_d[:, :])
            qsq = const.tile([B, D], DT.float32)
            ssq = const.tile([B, 1], DT.float32)
            nc.scalar.activation(qsq, q_sb, AF.Square, accum_out=ssq)
            lnq = const.tile([B, 1], DT.float32)
            nc.scalar.activation(lnq, ssq, AF.Ln, bias=eps12[:B])
            invq = const.tile([B, 1], DT.float32)
            nc.scalar.activation(invq, lnq, AF.Exp, scale=-0.5)
            qn = const.tile([B, D], DT.bfloat16)
            nc.vector.tensor_scalar_mul(out=qn, in0=q_sb, scalar1=invq)
            qnT = const.tile([P, 4, B], DT.bfloat16)
            for c in range(4):
                qt_ps = ps_wt.tile([P, B], DT.bfloat16, tag="qprep")
                nc.tensor.transpose(qt_ps, qn[:, c * P:(c + 1) * P], ident[:B, :B])
                nc.scalar.copy(qnT[:, c, :], qt_ps)

            # ---- main streaming loop ----
            acc_ps = ps_acc.tile([B, D], DT.float32)
            scr = dram_pool.tile([1, ntiles * P], DT.float32)
            slab_tiles = {}

            slab_ss = {}
            norms_done = [0]  # tiles with norms emitted (in order)

            def ensure_slab(sg):
                if sg in slab_tiles:
                    return slab_tiles[sg]
                t0, t1 = sg * g, min((sg + 1) * g, ntiles)  # tile range
                gg = t1 - t0
                a_sl = slab_pool.tile([P, gg, D], DT.bfloat16)
                last_slab = t1 == ntiles
                if not last_slab or real_last == P:
                    nc.gpsimd.dma_start(
                        out=a_sl,
                        in_=a_d[t0 * P:t1 * P, :].rearrange(
                            "(t p) d -> p t d", p=P))
                else:
                    for t in range(gg - 1):
                        r0 = (t0 + t) * P
                        nc.gpsimd.dma_start(out=a_sl[:, t, :], in_=a_d[r0:r0 + P, :])
                    nc.gpsimd.memset(a_sl[:, gg - 1, :], 0)
                    nc.gpsimd.dma_start(
                        out=a_sl[:real_last, gg - 1, :],
                        in_=a_d[(ntiles - 1) * P:npc, :])
                slab_tiles[sg] = a_sl
                return a_sl

            def a_tile(gt):
                sg, t = divmod(gt, g)
                return ensure_slab(sg)[:, t, :]

            def norm_tiles_upto(gt_end):
                """Emit per-tile norms (spread across quads) and, at each
                slab's last tile, the inv finalize + scr write."""
                while norms_done[0] < min(gt_end, ntiles):
                    gt = norms_done[0]
                    sg, t = divmod(gt, g)
                    t0, t1 = sg * g, min((sg + 1) * g, ntiles)
                    gg = t1 - t0
                    a_sl = ensure_slab(sg)
                    if t == 0:
                        ss_new = small.tile([P, gg], DT.float32, tag="ss")
                        slab_ss[sg] = ss_new
                    ss = slab_ss[sg]
                    sq = small.tile([P, D], DT.bfloat16, tag="sq")
                    if (gt % 8) < NORM_DVE_OF8:
                        nc.vector.affine_mul_reduce(
                            out=sq, accum_out=ss[:, t:t + 1],
                            in0=a_sl[:, t, :], in1=a_sl[:, t, :], scale=1.0,
                            bias=0.0)
                    else:
                        nc.scalar.activation(sq, a_sl[:, t, :], AF.Square,
                                             accum_out=ss[:, t:t + 1])
                    if t == gg - 1:
                        lns = small.tile([P, gg], DT.float32, tag="lns")
                        nc.scalar.activation(lns, ss, AF.Ln, bias=eps12)
                        inv = small.tile([P, gg], DT.float32, tag="inv")
                        nc.scalar.activation(inv, lns, AF.Exp, scale=-0.5,
                                             bias=ln10b)
                        ivt_ps = ps_wt.tile([g, P], DT.float32, tag="ivt")
                        nc.tensor.transpose(ivt_ps[:gg], inv, identf)
                        ivt = small.tile([g, P], DT.float32, tag="ivt_sb")
                        nc.vector.tensor_copy(ivt[:gg], ivt_ps[:gg])
                        nc.sync.dma_start(
                            out=bass.AP(tensor=scr.tensor,
                                        offset=scr.offset + t0 * P,
                                        ap=[[P, gg], [1, P]]),
                            in_=ivt[:gg])
                    norms_done[0] += 1


            def front_a(q):
                # A^T for the quad: [128d, 4c, 512n] via 16 PE transposes
                at_ps = ps_at.tile([P, 4, 4 * P], DT.bfloat16)
                for t in range(4):
                    a_t = a_tile(4 * q + t)
                    for c in range(4):
                        nc.tensor.transpose(
                            at_ps[:, c, t * P:(t + 1) * P],
                            a_t[:, c * P:(c + 1) * P], ident)
                at_sb = at_pool.tile([P, 4, 4 * P], DT.bfloat16)
                nc.vector.tensor_copy(at_sb, at_ps)
                # inv broadcast [64b, 512n] from DRAM scratch (HWDGE: keeps
                # the Q7 descriptor path free for the big slab loads)
                inv_bc = ssc_pool.tile([B, 4 * P], DT.float32, tag="inv_bc")
                nc.sync.dma_start(
                    out=inv_bc,
                    in_=bass.AP(tensor=scr.tensor,
                                offset=scr.offset + q * 4 * P,
                                ap=[[0, B], [1, 4 * P]]))
                return at_sb, inv_bc

            def front_b(q, at_sb, inv_bc):
                # sims: 4 wide matmuls accumulating over d-chunks
                s_ps = ps_s.tile([B, 4 * P], DT.float32, tag="s")
                for c in range(4):
                    nc.tensor.matmul(
                        s_ps, lhsT=qnT[:, c, :], rhs=at_sb[:, c, :],
                        start=(c == 0), stop=(c == 3))
                s_sc = ssc_pool.tile([B, 4 * P], DT.float32, tag="s_sc")
                nc.vector.tensor_mul(s_sc, s_ps, inv_bc)
                w_q = wt_pool.tile([B, 4 * P], DT.bfloat16, tag="w_q")
                nc.scalar.activation(w_q, s_sc, AF.Exp, bias=bias_main,
                                     accum_out=wsums[:, q:q + 1])
                return w_q

            def stage_back(q, w_q):
                wt_ps = ps_wt.tile([P, 4, B], DT.bfloat16, tag="wt")
                for t in range(4):
                    nc.tensor.transpose(
                        wt_ps[:, t, :], w_q[:, t * P:(t + 1) * P],
                        ident[:B, :B])
                wt_sb = wt_pool.tile([P, 4, B], DT.bfloat16, tag="wt_sb")
                nc.vector.tensor_copy(wt_sb, wt_ps)
                for t in range(4):
                    gt = 4 * q + t
                    nc.tensor.matmul(
                        acc_ps, lhsT=wt_sb[:, t, :], rhs=a_tile(gt),
                        start=(gt == 0), stop=(gt == ntiles - 1))

            # Software pipeline, skewed so every cross-engine dependency
            # has a full iteration of slack (no FIFO priority inversions):
            #   iter i emits: front_a(i) | back(i-2) | front_b(i-1)
            # PE stream/iter:  transp(i), wT(i-2), acc(i-2), sims(i-1)
            # DVE stream/iter: at-copy(i), wt-copy(i-2), mult(i-1), norms
            norm_tiles_upto(4 * NORM_AHEAD)
            fr = {}
            wq = {}
            BD = BACK_DEPTH
            for q in range(nquads + BD):
                if q < nquads:
                    sg_ahead = min((4 * (q + NORM_AHEAD)) // g + 1, nslabs - 1)
                    ensure_slab(sg_ahead)
                    fr[q] = front_a(q)
                if q - BD >= 0:
                    stage_back(q - BD, wq.pop(q - BD))
                if q < nquads:
                    norm_tiles_upto(4 * (q + 1 + NORM_AHEAD))
                if q - 1 >= 0 and q - 1 < nquads:
                    wq[q - 1] = front_b(q - 1, *fr.pop(q - 1))

            # ---- epilogue: writeback ----
            acc_sb = const.tile([B, D], DT.float32)
            nc.scalar.copy(acc_sb, acc_ps)
            nc.sync.dma_start(out=acc_d[:, :], in_=acc_sb)
            nc.sync.dma_start(out=lsum_d[:, :], in_=wsums)

    nc.finalize()
    return nc


_NC_CACHE = {}


def _get_nc(npc=NPC):
    if npc not in _NC_CACHE:
        _NC_CACHE[npc] = _build(npc)
    return _NC_CACHE[npc]


def kernel(query, addresses):
    global LAST_RESULTS
    query = np.ascontiguousarray(np.asarray(query), dtype=np.float32)
    addresses = np.ascontiguousarray(np.asarray(addresses), dtype=np.float32)
    n = addresses.shape[0]
    npc = n // NCORES
    assert npc * NCORES == n
    nc = _get_nc(npc)
    in_maps = [
        {"query": query, "addresses": addresses[c * npc:(c + 1) * npc]}
        for c in range(NCORES)
    ]
    res = run_bass_kernel_spmd(nc, in_maps, core_ids=list(range(NCORES)))
    LAST_RESULTS = res
    acc = np.zeros((B, D), np.float64)
    l = np.zeros((B, 1), np.float64)
    ntiles = (npc + P - 1) // P
    n_pad = ntiles * P - npc  # zero rows in the padded last tile
    for r in res.results:
        acc += r["acc"].astype(np.float64)
        l += r["lsum"].astype(np.float64).sum(axis=1, keepdims=True)
        if n_pad:
            # each pad row contributes exactly exp(0*scale - 10)
            l -= n_pad * math.exp(-10.0)
    return (acc / l).astype(np.float32)


# revision 20
# speedup vs baseline: 1.2769x; 1.0094x over previous
"""Trainium2 Bass kernel: cosine-similarity softmin retrieval (DSDM).

reference:  qn = q/||q||; an = a/||a||; sims = qn @ an^T            [B, N]
            w = softmax(10*sims) over N  (softmin of (1-sims)/0.1)
            out = (w @ A)                                           [B, D]

Strategy (8 NeuronCores, flash-attention-style split over N):
  - addresses [200000, 512] sharded row-wise, 25000 rows/core.
  - each core streams its shard once in 128-row tiles (bf16 on-chip, cast
    during the load DMA). Per quad of 4 tiles:
      * A^T chunks via 16 PE transposes -> at_ps [128d, 4c, 512n] (bf16 PSUM)
        -> one DVE copy to SBUF
      * s_raw [64b, 512n] via 4 PSUM-accumulated N=512 matmuls
        (lhsT = qn^T chunk, rhs = at chunk)
      * s_sc = s_raw * (10/||a||) on GPSIMD (inv broadcast along partitions
        via DRAM scratch roundtrip, per quad)
      * w = Exp(s_sc - 10) on ACT, accum_out -> per-quad wsum column
        (fixed shift: cos<=1 so logit-10 <= 0; no running max needed)
      * w^T via 4 PE transposes + DVE copy
      * acc [64, 512] += w^T.T @ A in PSUM across all tiles
  - row norms ss = sum(a^2) split DVE (affine_mul_reduce) / ACT (Square)
  - 10/||a|| = exp(-0.5*ln(ss + eps) + ln10) on ACT (one table set)
  - host: out = sum_c acc_c / sum_c l_c   (gather/unshard + tiny divide)

Padding: per-core row count 25000 = 195*128 + 40; the last tile's 88 pad
rows are zeroed; their s_sc is 0 so they get weight exp(-10), subtracted
exactly on the host.
"""

import math
import os
from collections import OrderedDict

import numpy as np

import concourse.bass as bass
import concourse.tile as tile
from concourse import bacc, mybir
from concourse.bass_utils import run_bass_kernel_spmd
from concourse.masks import make_identity

DT = mybir.dt
AF = mybir.ActivationFunctionType
ALU = mybir.AluOpType

B = 64
D = 512
N_FULL = 200000
NCORES = 8
NPC = N_FULL // NCORES  # 25000
P = 128
LN10 = math.log(10.0)

G = int(os.environ.get("KERNEL_G", "14"))  # tiles per DMA slab
FP8 = os.environ.get("KERNEL_FP8", "0") == "1"
QN8 = os.environ.get("KERNEL_QN8", "1") == "1"  # qnT in fp8 (DoubleRow sims)
NORM_DVE_OF8 = int(os.environ.get("KERNEL_NORM_DVE_OF8", "3"))  # tiles/8 on DVE
SLAB_BUFS = int(os.environ.get("KERNEL_SLAB_BUFS", "6"))
NORM_AHEAD = int(os.environ.get("KERNEL_NORM_AHEAD", "5"))  # quads of norm lead
MULT_ENGINE = os.environ.get("KERNEL_MULT", "dve")
BACK_DEPTH = int(os.environ.get("KERNEL_BACK_DEPTH", "3"))

LAST_RESULTS = None  # test harness reads exec_time_ns from here


def _patch_act_tables():
    """Prefer the combined natural_log_exp set so Ln/Exp/Square/Copy share
    one ACT table load instead of thrashing 2 loads per slab (~2.7us each)."""
    if getattr(bacc.get_activation_tables, "_patched", False):
        return
    orig = bacc.get_activation_tables

    keep = {AF.Ln, AF.Exp, AF.Square}

    def patched(arch):
        tabs = orig(arch)
        out = OrderedDict()
        for k, fns in tabs.items():
            if k == "natural_log_exp_and_others":
                out[k] = fns
            else:
                out[k] = {f for f in fns if f not in keep}
        return out

    patched._patched = True
    bacc.get_activation_tables = patched


def _build(npc=NPC):
    _patch_act_tables()
    ntiles = (npc + P - 1) // P
    assert ntiles % 4 == 0
    nquads = ntiles // 4
    g = G
    nslabs = (ntiles + g - 1) // g
    real_last = npc - (ntiles - 1) * P  # rows in final tile

    adt = DT.float8e4 if FP8 else DT.bfloat16
    exp_bias = -10.0  # w is bf16 in both modes; logit-10 <= 0 so w <= 1
    nc = bacc.Bacc("TRN2")
    q_d = nc.dram_tensor("query", [B, D], DT.float32, kind="ExternalInput")
    a_d = nc.dram_tensor("addresses", [npc, D], DT.float32, kind="ExternalInput")
    acc_d = nc.dram_tensor("acc", [B, D], DT.float32, kind="ExternalOutput")
    lsum_d = nc.dram_tensor("lsum", [B, nquads], DT.float32, kind="ExternalOutput")

    with tile.TileContext(nc) as tc:
        with (
            tc.tile_pool(name="const", bufs=1) as const,
            tc.tile_pool(name="slab", bufs=SLAB_BUFS) as slab_pool,
            tc.tile_pool(name="at", bufs=2) as at_pool,
            tc.tile_pool(name="wt", bufs=5) as wt_pool,
            tc.tile_pool(name="ssc", bufs=2) as ssc_pool,
            tc.tile_pool(name="small", bufs=4) as small,
            tc.tile_pool(name="ps_at", bufs=1, space="PSUM") as ps_at,
            tc.tile_pool(name="ps_s", bufs=2, space="PSUM") as ps_s,
            tc.tile_pool(name="ps_wt", bufs=1, space="PSUM") as ps_wt,
            tc.tile_pool(name="ps_acc", bufs=1, space="PSUM") as ps_acc,
            tc.tile_pool(name="dram", bufs=1, space="DRAM") as dram_pool,
        ):
            ident = const.tile([P, P], adt)
            make_identity(nc, ident)
            identb = const.tile([B, B], DT.bfloat16)
            make_identity(nc, identb)
            bias_main = const.tile([B, 1], DT.float32)
            nc.vector.memset(bias_main, exp_bias)
            eps12 = const.tile([P, 1], DT.float32)
            nc.vector.memset(eps12, 1e-12)
            ln10b = const.tile([P, 1], DT.float32)
            nc.vector.memset(ln10b, LN10)
            identf = const.tile([P, P], DT.float32)
            make_identity(nc, identf)
            wsums = const.tile([B, nquads], DT.float32)

            # ---- query preprocessing: qn^T bf16 chunks [128d, 4c, 64b] ----
            q_sb = const.tile([B, D], DT.float32)
            nc.sync.dma_start(out=q_sb, in_=q_d[:, :])
            qsq = const.tile([B, D], DT.float32)
            ssq = const.tile([B, 1], DT.float32)
            nc.scalar.activation(qsq, q_sb, AF.Square, accum_out=ssq)
            lnq = const.tile([B, 1], DT.float32)
            nc.scalar.activation(lnq, ssq, AF.Ln, bias=eps12[:B])
            invq = const.tile([B, 1], DT.float32)
            nc.scalar.activation(invq, lnq, AF.Exp, scale=-0.5)
            qdt = adt if QN8 else DT.bfloat16
            qn = const.tile([B, D], qdt)
            nc.vector.tensor_scalar_mul(out=qn, in0=q_sb, scalar1=invq)
            qnT = const.tile([P, 4, B], qdt)
            for c in range(4):
                if FP8 and QN8:
                    qt_ps = ps_wt.tile([P, B, 2], adt, tag="qprep")
                    nc.tensor.transpose(qt_ps[:, :, 0],
                                        qn[:, c * P:(c + 1) * P], ident[:B, :B])
                    nc.vector.tensor_copy(qnT[:, c, :], qt_ps[:, :, 0])
                else:
                    qt_ps = ps_wt.tile([P, B], qdt, tag="qprep")
                    nc.tensor.transpose(qt_ps, qn[:, c * P:(c + 1) * P],
                                        identb if FP8 else ident[:B, :B])
                    nc.scalar.copy(qnT[:, c, :], qt_ps)

            # ---- main streaming loop ----
            acc_ps = ps_acc.tile([B, D], DT.float32)
            scr = dram_pool.tile([1, ntiles * P], DT.float32)
            slab_tiles = {}

            slab_ss = {}
            norms_done = [0]  # tiles with norms emitted (in order)

            def ensure_slab(sg):
                if sg in slab_tiles:
                    return slab_tiles[sg]
                t0, t1 = sg * g, min((sg + 1) * g, ntiles)  # tile range
                gg = t1 - t0
                a_sl = slab_pool.tile([P, gg, D], adt)
                last_slab = t1 == ntiles
                if not last_slab or real_last == P:
                    nc.gpsimd.dma_start(
                        out=a_sl,
                        in_=a_d[t0 * P:t1 * P, :].rearrange(
                            "(t p) d -> p t d", p=P))
                else:
                    for t in range(gg - 1):
                        r0 = (t0 + t) * P
                        nc.gpsimd.dma_start(out=a_sl[:, t, :], in_=a_d[r0:r0 + P, :])
                    nc.gpsimd.memset(a_sl[:, gg - 1, :], 0)
                    nc.gpsimd.dma_start(
                        out=a_sl[:real_last, gg - 1, :],
                        in_=a_d[(ntiles - 1) * P:npc, :])
                slab_tiles[sg] = a_sl
                return a_sl

            def a_tile(gt):
                sg, t = divmod(gt, g)
                return ensure_slab(sg)[:, t, :]

            def norm_tiles_upto(gt_end):
                """Emit per-tile norms (spread across quads) and, at each
                slab's last tile, the inv finalize + scr write."""
                while norms_done[0] < min(gt_end, ntiles):
                    gt = norms_done[0]
                    sg, t = divmod(gt, g)
                    t0, t1 = sg * g, min((sg + 1) * g, ntiles)
                    gg = t1 - t0
                    a_sl = ensure_slab(sg)
                    if t == 0:
                        ss_new = small.tile([P, gg], DT.float32, tag="ss")
                        slab_ss[sg] = ss_new
                    ss = slab_ss[sg]
                    sq = small.tile([P, D], DT.bfloat16, tag="sq")
                    if (gt % 8) < NORM_DVE_OF8:
                        nc.vector.affine_mul_reduce(
                            out=sq, accum_out=ss[:, t:t + 1],
                            in0=a_sl[:, t, :], in1=a_sl[:, t, :], scale=1.0,
                            bias=0.0)
                    else:
                        nc.scalar.activation(sq, a_sl[:, t, :], AF.Square,
                                             accum_out=ss[:, t:t + 1])
                    if t == gg - 1:
                        lns = small.tile([P, gg], DT.float32, tag="lns")
                        nc.scalar.activation(lns, ss, AF.Ln, bias=eps12)
                        inv = small.tile([P, gg], DT.float32, tag="inv")
                        nc.scalar.activation(inv, lns, AF.Exp, scale=-0.5,
                                             bias=ln10b)
                        ivt_ps = ps_wt.tile([g, P], DT.float32, tag="ivt")
                        nc.tensor.transpose(ivt_ps[:gg], inv, identf)
                        ivt = small.tile([g, P], DT.float32, tag="ivt_sb")
                        nc.vector.tensor_copy(ivt[:gg], ivt_ps[:gg])
                        nc.sync.dma_start(
                            out=bass.AP(tensor=scr.tensor,
                                        offset=scr.offset + t0 * P,
                                        ap=[[P, gg], [1, P]]),
                            in_=ivt[:gg])
                    norms_done[0] += 1


            def front_a(q):
                # A^T for the quad: [128d, 4c, 512n] via 16 PE transposes.
                # fp8 transposes write 16-bit granules (value in the low byte)
                # so the PSUM tile carries a trailing pair dim; the SBUF copy
                # moves dense uint16 and sims reads the fp8 view with step 2.
                if FP8:
                    at_ps = ps_at.tile([P, 4, 4 * P, 2], adt)
                    for t in range(4):
                        a_t = a_tile(4 * q + t)
                        for c in range(4):
                            nc.tensor.transpose(
                                at_ps[:, c, t * P:(t + 1) * P, 0],
                                a_t[:, c * P:(c + 1) * P], ident)
                    at_sb = at_pool.tile([P, 4, 4 * P], DT.uint16)
                    nc.vector.tensor_copy(at_sb,
                                          at_ps.bitcast(DT.uint16)[:, :, :, 0])
                else:
                    at_ps = ps_at.tile([P, 4, 4 * P], adt)
                    for t in range(4):
                        a_t = a_tile(4 * q + t)
                        for c in range(4):
                            nc.tensor.transpose(
                                at_ps[:, c, t * P:(t + 1) * P],
                                a_t[:, c * P:(c + 1) * P], ident)
                    at_sb = at_pool.tile([P, 4, 4 * P], adt)
                    nc.vector.tensor_copy(at_sb, at_ps)
                # inv broadcast [64b, 512n] from DRAM scratch (HWDGE: keeps
                # the Q7 descriptor path free for the big slab loads)
                inv_bc = ssc_pool.tile([B, 4 * P], DT.float32, tag="inv_bc")
                nc.sync.dma_start(
                    out=inv_bc,
                    in_=bass.AP(tensor=scr.tensor,
                                offset=scr.offset + q * 4 * P,
                                ap=[[0, B], [1, 4 * P]]))
                return at_sb, inv_bc

            def front_b(q, at_sb, inv_bc):
                # sims: 4 wide matmuls accumulating over d-chunks
                s_ps = ps_s.tile([B, 4 * P], DT.float32, tag="s")
                if FP8 and QN8:
                    at8 = at_sb.bitcast(DT.float8e4).rearrange(
                        "p k (n two) -> p k n two", two=2)
                    for j in range(2):
                        nc.tensor.matmul(
                            s_ps, lhsT=qnT[:, 2 * j:2 * j + 2, :],
                            rhs=at8[:, 2 * j:2 * j + 2, :, 0],
                            start=(j == 0), stop=(j == 1),
                            perf_mode=mybir.MatmulPerfMode.DoubleRow)
                elif FP8:
                    at8 = at_sb.bitcast(DT.float8e4).rearrange(
                        "p k (n two) -> p k n two", two=2)
                    for c in range(4):
                        nc.tensor.matmul(
                            s_ps, lhsT=qnT[:, c, :], rhs=at8[:, c, :, 0],
                            start=(c == 0), stop=(c == 3))
                else:
                    for c in range(4):
                        nc.tensor.matmul(
                            s_ps, lhsT=qnT[:, c, :], rhs=at_sb[:, c, :],
                            start=(c == 0), stop=(c == 3))
                s_sc = ssc_pool.tile([B, 4 * P], DT.float32, tag="s_sc")
                nc.vector.tensor_mul(s_sc, s_ps, inv_bc)
                w_q = wt_pool.tile([B, 4 * P], DT.bfloat16, tag="w_q")
                nc.scalar.activation(w_q, s_sc, AF.Exp, bias=bias_main,
                                     accum_out=wsums[:, q:q + 1])
                return w_q

            def stage_back(q, w_q):
                # w stays bf16 (fp8 weights lose too much mantissa); the acc
                # matmuls mix bf16 lhsT with the fp8 A stream (HW-exact).
                wt_ps = ps_wt.tile([P, 4, B], DT.bfloat16, tag="wt")
                for t in range(4):
                    nc.tensor.transpose(
                        wt_ps[:, t, :], w_q[:, t * P:(t + 1) * P], identb)
                wt_sb = wt_pool.tile([P, 4, B], DT.bfloat16, tag="wt_sb")
                nc.vector.tensor_copy(wt_sb, wt_ps)
                for t in range(4):
                    gt = 4 * q + t
                    nc.tensor.matmul(
                        acc_ps, lhsT=wt_sb[:, t, :], rhs=a_tile(gt),
                        start=(gt == 0), stop=(gt == ntiles - 1))

            # Software pipeline, skewed so every cross-engine dependency
            # has a full iteration of slack (no FIFO priority inversions):
            #   iter i emits: front_a(i) | back(i-2) | front_b(i-1)
            # PE stream/iter:  transp(i), wT(i-2), acc(i-2), sims(i-1)
            # DVE stream/iter: at-copy(i), wt-copy(i-2), mult(i-1), norms
            norm_tiles_upto(4 * NORM_AHEAD)
            fr = {}
            wq = {}
            BD = BACK_DEPTH
            for q in range(nquads + BD):
                if q < nquads:
                    sg_ahead = min((4 * (q + NORM_AHEAD)) // g + 1, nslabs - 1)
                    ensure_slab(sg_ahead)
                    fr[q] = front_a(q)
                if q - BD >= 0:
                    stage_back(q - BD, wq.pop(q - BD))
                if q < nquads:
                    norm_tiles_upto(4 * (q + 1 + NORM_AHEAD))
                if q - 1 >= 0 and q - 1 < nquads:
                    wq[q - 1] = front_b(q - 1, *fr.pop(q - 1))

            # ---- epilogue: writeback ----
            acc_sb = const.tile([B, D], DT.float32)
            nc.scalar.copy(acc_sb, acc_ps)
            nc.sync.dma_start(out=acc_d[:, :], in_=acc_sb)
            nc.sync.dma_start(out=lsum_d[:, :], in_=wsums)

    nc.finalize()
    return nc


_NC_CACHE = {}


def _get_nc(npc=NPC):
    if npc not in _NC_CACHE:
        _NC_CACHE[npc] = _build(npc)
    return _NC_CACHE[npc]


EXP_BIAS = -10.0


def kernel(query, addresses):
    global LAST_RESULTS
    query = np.ascontiguousarray(np.asarray(query), dtype=np.float32)
    addresses = np.ascontiguousarray(np.asarray(addresses), dtype=np.float32)
    n = addresses.shape[0]
    npc = n // NCORES
    assert npc * NCORES == n
    nc = _get_nc(npc)
    in_maps = [
        {"query": query, "addresses": addresses[c * npc:(c + 1) * npc]}
        for c in range(NCORES)
    ]
    res = run_bass_kernel_spmd(nc, in_maps, core_ids=list(range(NCORES)))
    LAST_RESULTS = res
    acc = np.zeros((B, D), np.float64)
    l = np.zeros((B, 1), np.float64)
    ntiles = (npc + P - 1) // P
    n_pad = ntiles * P - npc  # zero rows in the padded last tile
    for r in res.results:
        acc += r["acc"].astype(np.float64)
        l += r["lsum"].astype(np.float64).sum(axis=1, keepdims=True)
        if n_pad:
            # each pad row contributes exactly exp(0*scale + bias)
            l -= n_pad * math.exp(EXP_BIAS)
    return (acc / l).astype(np.float32)


# revision 21
# speedup vs baseline: 1.2948x; 1.0140x over previous
"""Trainium2 Bass kernel: cosine-similarity softmin retrieval (DSDM).

reference:  qn = q/||q||; an = a/||a||; sims = qn @ an^T            [B, N]
            w = softmax(10*sims) over N  (softmin of (1-sims)/0.1)
            out = (w @ A)                                           [B, D]

Strategy (8 NeuronCores, flash-attention-style split over N):
  - addresses [200000, 512] sharded row-wise, 25000 rows/core.
  - each core streams its shard once in 128-row tiles (bf16 on-chip, cast
    during the load DMA). Per quad of 4 tiles:
      * A^T chunks via 16 PE transposes -> at_ps [128d, 4c, 512n] (bf16 PSUM)
        -> one DVE copy to SBUF
      * s_raw [64b, 512n] via 4 PSUM-accumulated N=512 matmuls
        (lhsT = qn^T chunk, rhs = at chunk)
      * s_sc = s_raw * (10/||a||) on GPSIMD (inv broadcast along partitions
        via DRAM scratch roundtrip, per quad)
      * w = Exp(s_sc - 10) on ACT, accum_out -> per-quad wsum column
        (fixed shift: cos<=1 so logit-10 <= 0; no running max needed)
      * w^T via 4 PE transposes + DVE copy
      * acc [64, 512] += w^T.T @ A in PSUM across all tiles
  - row norms ss = sum(a^2) split DVE (affine_mul_reduce) / ACT (Square)
  - 10/||a|| = exp(-0.5*ln(ss + eps) + ln10) on ACT (one table set)
  - host: out = sum_c acc_c / sum_c l_c   (gather/unshard + tiny divide)

Padding: per-core row count 25000 = 195*128 + 40; the last tile's 88 pad
rows are zeroed; their s_sc is 0 so they get weight exp(-10), subtracted
exactly on the host.
"""

import math
import os
from collections import OrderedDict

import numpy as np

import concourse.bass as bass
import concourse.tile as tile
from concourse import bacc, mybir
from concourse.bass_utils import run_bass_kernel_spmd
from concourse.masks import make_identity

DT = mybir.dt
AF = mybir.ActivationFunctionType
ALU = mybir.AluOpType

B = 64
D = 512
N_FULL = 200000
NCORES = 8
NPC = N_FULL // NCORES  # 25000
P = 128
LN10 = math.log(10.0)

G = int(os.environ.get("KERNEL_G", "14"))  # tiles per DMA slab
FP8 = os.environ.get("KERNEL_FP8", "0") == "1"
W8 = os.environ.get("KERNEL_W8", "1") == "1"  # fp8 weights + DoubleRow acc
QN8 = os.environ.get("KERNEL_QN8", "1") == "1"  # qnT in fp8 (DoubleRow sims)
NORM_DVE_OF8 = int(os.environ.get("KERNEL_NORM_DVE_OF8", "3"))  # tiles/8 on DVE
SLAB_BUFS = int(os.environ.get("KERNEL_SLAB_BUFS", "6"))
NORM_AHEAD = int(os.environ.get("KERNEL_NORM_AHEAD", "5"))  # quads of norm lead
MULT_ENGINE = os.environ.get("KERNEL_MULT", "dve")
BACK_DEPTH = int(os.environ.get("KERNEL_BACK_DEPTH", "3"))

LAST_RESULTS = None  # test harness reads exec_time_ns from here


def _patch_act_tables():
    """Prefer the combined natural_log_exp set so Ln/Exp/Square/Copy share
    one ACT table load instead of thrashing 2 loads per slab (~2.7us each)."""
    if getattr(bacc.get_activation_tables, "_patched", False):
        return
    orig = bacc.get_activation_tables

    keep = {AF.Ln, AF.Exp, AF.Square}

    def patched(arch):
        tabs = orig(arch)
        out = OrderedDict()
        for k, fns in tabs.items():
            if k == "natural_log_exp_and_others":
                out[k] = fns
            else:
                out[k] = {f for f in fns if f not in keep}
        return out

    patched._patched = True
    bacc.get_activation_tables = patched


def _build(npc=NPC):
    _patch_act_tables()
    ntiles = (npc + P - 1) // P
    assert ntiles % 4 == 0
    nquads = ntiles // 4
    g = G
    nslabs = (ntiles + g - 1) // g
    real_last = npc - (ntiles - 1) * P  # rows in final tile

    adt = DT.float8e4 if FP8 else DT.bfloat16
    # fixed softmax shift. bf16 w: -10 (logit-10 <= 0 so w <= 1). fp8 w:
    # -2 keeps typical weights e^(-2±0.5) in e4m3's NORMAL range (>= 2^-6,
    # ~3.6% rms); data max logit is 2.3 (max w 1.3), overflow would need
    # cos > 0.81 vs the observed 0.23.
    w8 = FP8 and W8
    exp_bias = -2.0 if w8 else -10.0
    nc = bacc.Bacc("TRN2")
    q_d = nc.dram_tensor("query", [B, D], DT.float32, kind="ExternalInput")
    a_d = nc.dram_tensor("addresses", [npc, D], DT.float32, kind="ExternalInput")
    acc_d = nc.dram_tensor("acc", [B, D], DT.float32, kind="ExternalOutput")
    lsum_d = nc.dram_tensor("lsum", [B, nquads], DT.float32, kind="ExternalOutput")

    with tile.TileContext(nc) as tc:
        with (
            tc.tile_pool(name="const", bufs=1) as const,
            tc.tile_pool(name="slab", bufs=SLAB_BUFS) as slab_pool,
            tc.tile_pool(name="at", bufs=2) as at_pool,
            tc.tile_pool(name="wt", bufs=5) as wt_pool,
            tc.tile_pool(name="ssc", bufs=2) as ssc_pool,
            tc.tile_pool(name="small", bufs=4) as small,
            tc.tile_pool(name="ps_at", bufs=1, space="PSUM") as ps_at,
            tc.tile_pool(name="ps_s", bufs=2, space="PSUM") as ps_s,
            tc.tile_pool(name="ps_wt", bufs=1, space="PSUM") as ps_wt,
            tc.tile_pool(name="ps_acc", bufs=1, space="PSUM") as ps_acc,
            tc.tile_pool(name="dram", bufs=1, space="DRAM") as dram_pool,
        ):
            ident = const.tile([P, P], adt)
            make_identity(nc, ident)
            identb = const.tile([B, B], DT.bfloat16)
            make_identity(nc, identb)
            bias_main = const.tile([B, 1], DT.float32)
            nc.vector.memset(bias_main, exp_bias)
            eps12 = const.tile([P, 1], DT.float32)
            nc.vector.memset(eps12, 1e-12)
            ln10b = const.tile([P, 1], DT.float32)
            nc.vector.memset(ln10b, LN10)
            identf = const.tile([P, P], DT.float32)
            make_identity(nc, identf)
            wsums = const.tile([B, nquads], DT.float32)

            # ---- query preprocessing: qn^T bf16 chunks [128d, 4c, 64b] ----
            q_sb = const.tile([B, D], DT.float32)
            nc.sync.dma_start(out=q_sb, in_=q_d[:, :])
            qsq = const.tile([B, D], DT.float32)
            ssq = const.tile([B, 1], DT.float32)
            nc.scalar.activation(qsq, q_sb, AF.Square, accum_out=ssq)
            lnq = const.tile([B, 1], DT.float32)
            nc.scalar.activation(lnq, ssq, AF.Ln, bias=eps12[:B])
            invq = const.tile([B, 1], DT.float32)
            nc.scalar.activation(invq, lnq, AF.Exp, scale=-0.5)
            qdt = adt if QN8 else DT.bfloat16
            qn = const.tile([B, D], qdt)
            nc.vector.tensor_scalar_mul(out=qn, in0=q_sb, scalar1=invq)
            qnT = const.tile([P, 4, B], qdt)
            for c in range(4):
                if FP8 and QN8:
                    qt_ps = ps_wt.tile([P, B, 2], adt, tag="qprep")
                    nc.tensor.transpose(qt_ps[:, :, 0],
                                        qn[:, c * P:(c + 1) * P], ident[:B, :B])
                    nc.vector.tensor_copy(qnT[:, c, :], qt_ps[:, :, 0])
                else:
                    qt_ps = ps_wt.tile([P, B], qdt, tag="qprep")
                    nc.tensor.transpose(qt_ps, qn[:, c * P:(c + 1) * P],
                                        identb if FP8 else ident[:B, :B])
                    nc.scalar.copy(qnT[:, c, :], qt_ps)

            # ---- main streaming loop ----
            acc_ps = ps_acc.tile([B, D], DT.float32)
            scr = dram_pool.tile([1, ntiles * P], DT.float32)
            slab_tiles = {}

            slab_ss = {}
            norms_done = [0]  # tiles with norms emitted (in order)

            def ensure_slab(sg):
                if sg in slab_tiles:
                    return slab_tiles[sg]
                t0, t1 = sg * g, min((sg + 1) * g, ntiles)  # tile range
                gg = t1 - t0
                a_sl = slab_pool.tile([P, gg, D], adt)
                last_slab = t1 == ntiles
                if not last_slab or real_last == P:
                    nc.gpsimd.dma_start(
                        out=a_sl,
                        in_=a_d[t0 * P:t1 * P, :].rearrange(
                            "(t p) d -> p t d", p=P))
                else:
                    for t in range(gg - 1):
                        r0 = (t0 + t) * P
                        nc.gpsimd.dma_start(out=a_sl[:, t, :], in_=a_d[r0:r0 + P, :])
                    nc.gpsimd.memset(a_sl[:, gg - 1, :], 0)
                    nc.gpsimd.dma_start(
                        out=a_sl[:real_last, gg - 1, :],
                        in_=a_d[(ntiles - 1) * P:npc, :])
                slab_tiles[sg] = a_sl
                return a_sl

            def a_tile(gt):
                sg, t = divmod(gt, g)
                return ensure_slab(sg)[:, t, :]

            def norm_tiles_upto(gt_end):
                """Emit per-tile norms (spread across quads) and, at each
                slab's last tile, the inv finalize + scr write."""
                while norms_done[0] < min(gt_end, ntiles):
                    gt = norms_done[0]
                    sg, t = divmod(gt, g)
                    t0, t1 = sg * g, min((sg + 1) * g, ntiles)
                    gg = t1 - t0
                    a_sl = ensure_slab(sg)
                    if t == 0:
                        ss_new = small.tile([P, gg], DT.float32, tag="ss")
                        slab_ss[sg] = ss_new
                    ss = slab_ss[sg]
                    sq = small.tile([P, D], DT.bfloat16, tag="sq")
                    if (gt % 8) < NORM_DVE_OF8:
                        nc.vector.affine_mul_reduce(
                            out=sq, accum_out=ss[:, t:t + 1],
                            in0=a_sl[:, t, :], in1=a_sl[:, t, :], scale=1.0,
                            bias=0.0)
                    else:
                        nc.scalar.activation(sq, a_sl[:, t, :], AF.Square,
                                             accum_out=ss[:, t:t + 1])
                    if t == gg - 1:
                        lns = small.tile([P, gg], DT.float32, tag="lns")
                        nc.scalar.activation(lns, ss, AF.Ln, bias=eps12)
                        inv = small.tile([P, gg], DT.float32, tag="inv")
                        nc.scalar.activation(inv, lns, AF.Exp, scale=-0.5,
                                             bias=ln10b)
                        ivt_ps = ps_wt.tile([g, P], DT.float32, tag="ivt")
                        nc.tensor.transpose(ivt_ps[:gg], inv, identf)
                        ivt = small.tile([g, P], DT.float32, tag="ivt_sb")
                        nc.vector.tensor_copy(ivt[:gg], ivt_ps[:gg])
                        nc.sync.dma_start(
                            out=bass.AP(tensor=scr.tensor,
                                        offset=scr.offset + t0 * P,
                                        ap=[[P, gg], [1, P]]),
                            in_=ivt[:gg])
                    norms_done[0] += 1


            def front_a(q):
                # A^T for the quad: [128d, 4c, 512n] via 16 PE transposes.
                # fp8 transposes write 16-bit granules (value in the low byte)
                # so the PSUM tile carries a trailing pair dim; the SBUF copy
                # moves dense uint16 and sims reads the fp8 view with step 2.
                if FP8:
                    at_ps = ps_at.tile([P, 4, 4 * P, 2], adt)
                    for t in range(4):
                        a_t = a_tile(4 * q + t)
                        for c in range(4):
                            nc.tensor.transpose(
                                at_ps[:, c, t * P:(t + 1) * P, 0],
                                a_t[:, c * P:(c + 1) * P], ident)
                    at_sb = at_pool.tile([P, 4, 4 * P], DT.uint16)
                    nc.vector.tensor_copy(at_sb,
                                          at_ps.bitcast(DT.uint16)[:, :, :, 0])
                else:
                    at_ps = ps_at.tile([P, 4, 4 * P], adt)
                    for t in range(4):
                        a_t = a_tile(4 * q + t)
                        for c in range(4):
                            nc.tensor.transpose(
                                at_ps[:, c, t * P:(t + 1) * P],
                                a_t[:, c * P:(c + 1) * P], ident)
                    at_sb = at_pool.tile([P, 4, 4 * P], adt)
                    nc.vector.tensor_copy(at_sb, at_ps)
                # inv broadcast [64b, 512n] from DRAM scratch (HWDGE: keeps
                # the Q7 descriptor path free for the big slab loads)
                inv_bc = ssc_pool.tile([B, 4 * P], DT.float32, tag="inv_bc")
                nc.sync.dma_start(
                    out=inv_bc,
                    in_=bass.AP(tensor=scr.tensor,
                                offset=scr.offset + q * 4 * P,
                                ap=[[0, B], [1, 4 * P]]))
                return at_sb, inv_bc

            def front_b(q, at_sb, inv_bc):
                # sims: 4 wide matmuls accumulating over d-chunks
                s_ps = ps_s.tile([B, 4 * P], DT.float32, tag="s")
                if FP8 and QN8:
                    at8 = at_sb.bitcast(DT.float8e4).rearrange(
                        "p k (n two) -> p k n two", two=2)
                    for j in range(2):
                        nc.tensor.matmul(
                            s_ps, lhsT=qnT[:, 2 * j:2 * j + 2, :],
                            rhs=at8[:, 2 * j:2 * j + 2, :, 0],
                            start=(j == 0), stop=(j == 1),
                            perf_mode=mybir.MatmulPerfMode.DoubleRow)
                elif FP8:
                    at8 = at_sb.bitcast(DT.float8e4).rearrange(
                        "p k (n two) -> p k n two", two=2)
                    for c in range(4):
                        nc.tensor.matmul(
                            s_ps, lhsT=qnT[:, c, :], rhs=at8[:, c, :, 0],
                            start=(c == 0), stop=(c == 3))
                else:
                    for c in range(4):
                        nc.tensor.matmul(
                            s_ps, lhsT=qnT[:, c, :], rhs=at_sb[:, c, :],
                            start=(c == 0), stop=(c == 3))
                s_sc = ssc_pool.tile([B, 4 * P], DT.float32, tag="s_sc")
                nc.vector.tensor_mul(s_sc, s_ps, inv_bc)
                w_q = wt_pool.tile([B, 4 * P],
                                   adt if w8 else DT.bfloat16, tag="w_q")
                nc.scalar.activation(w_q, s_sc, AF.Exp, bias=bias_main,
                                     accum_out=wsums[:, q:q + 1])
                return w_q

            def stage_back(q, w_q):
                if w8:
                    wt_ps = ps_wt.tile([P, 4, B, 2], adt, tag="wt")
                    for t in range(4):
                        nc.tensor.transpose(
                            wt_ps[:, t, :, 0], w_q[:, t * P:(t + 1) * P],
                            ident[:B, :B])
                    wt_sb = wt_pool.tile([P, 4, B], adt, tag="wt_sb")
                    nc.vector.tensor_copy(wt_sb, wt_ps[:, :, :, 0])
                    for j in range(2):
                        gt = 4 * q + 2 * j
                        sg, t = divmod(gt, g)
                        a_sl = ensure_slab(sg)
                        nc.tensor.matmul(
                            acc_ps, lhsT=wt_sb[:, 2 * j:2 * j + 2, :],
                            rhs=a_sl[:, t:t + 2, :],
                            start=(gt == 0), stop=(gt == ntiles - 2),
                            perf_mode=mybir.MatmulPerfMode.DoubleRow)
                else:
                    wt_ps = ps_wt.tile([P, 4, B], DT.bfloat16, tag="wt")
                    for t in range(4):
                        nc.tensor.transpose(
                            wt_ps[:, t, :], w_q[:, t * P:(t + 1) * P], identb)
                    wt_sb = wt_pool.tile([P, 4, B], DT.bfloat16, tag="wt_sb")
                    nc.vector.tensor_copy(wt_sb, wt_ps)
                    for t in range(4):
                        gt = 4 * q + t
                        nc.tensor.matmul(
                            acc_ps, lhsT=wt_sb[:, t, :], rhs=a_tile(gt),
                            start=(gt == 0), stop=(gt == ntiles - 1))

            # Software pipeline, skewed so every cross-engine dependency
            # has a full iteration of slack (no FIFO priority inversions):
            #   iter i emits: front_a(i) | back(i-2) | front_b(i-1)
            # PE stream/iter:  transp(i), wT(i-2), acc(i-2), sims(i-1)
            # DVE stream/iter: at-copy(i), wt-copy(i-2), mult(i-1), norms
            norm_tiles_upto(4 * NORM_AHEAD)
            fr = {}
            wq = {}
            BD = BACK_DEPTH
            for q in range(nquads + BD):
                if q < nquads:
                    sg_ahead = min((4 * (q + NORM_AHEAD)) // g + 1, nslabs - 1)
                    ensure_slab(sg_ahead)
                    fr[q] = front_a(q)
                if q - BD >= 0:
                    stage_back(q - BD, wq.pop(q - BD))
                if q < nquads:
                    norm_tiles_upto(4 * (q + 1 + NORM_AHEAD))
                if q - 1 >= 0 and q - 1 < nquads:
                    wq[q - 1] = front_b(q - 1, *fr.pop(q - 1))

            # ---- epilogue: writeback ----
            acc_sb = const.tile([B, D], DT.float32)
            nc.scalar.copy(acc_sb, acc_ps)
            nc.sync.dma_start(out=acc_d[:, :], in_=acc_sb)
            nc.sync.dma_start(out=lsum_d[:, :], in_=wsums)

    nc.finalize()
    return nc


_NC_CACHE = {}


def _get_nc(npc=NPC):
    if npc not in _NC_CACHE:
        _NC_CACHE[npc] = _build(npc)
    return _NC_CACHE[npc]


EXP_BIAS = -2.0 if (FP8 and os.environ.get("KERNEL_W8", "1") == "1") else -10.0


def kernel(query, addresses):
    global LAST_RESULTS
    query = np.ascontiguousarray(np.asarray(query), dtype=np.float32)
    addresses = np.ascontiguousarray(np.asarray(addresses), dtype=np.float32)
    n = addresses.shape[0]
    npc = n // NCORES
    assert npc * NCORES == n
    nc = _get_nc(npc)
    in_maps = [
        {"query": query, "addresses": addresses[c * npc:(c + 1) * npc]}
        for c in range(NCORES)
    ]
    res = run_bass_kernel_spmd(nc, in_maps, core_ids=list(range(NCORES)))
    LAST_RESULTS = res
    acc = np.zeros((B, D), np.float64)
    l = np.zeros((B, 1), np.float64)
    ntiles = (npc + P - 1) // P
    n_pad = ntiles * P - npc  # zero rows in the padded last tile
    for r in res.results:
        acc += r["acc"].astype(np.float64)
        l += r["lsum"].astype(np.float64).sum(axis=1, keepdims=True)
        if n_pad:
            # each pad row contributes exactly exp(0*scale + bias)
            l -= n_pad * math.exp(EXP_BIAS)
    return (acc / l).astype(np.float32)
